# revision 33
# baseline (speedup 1.0000x reference)
"""Multi-head self-attention (inverted causal mask) on 8 Trainium2 cores.

Problem: B=2, P=2048 seq, M=1024 model dim, N=16 heads, H=64 head dim.
Sharding: data-parallel on batch (2) x tensor-parallel on heads (4 groups
of 4 heads) = 8 cores. Each core computes, for its batch b and its 4 heads,
the full attention pipeline and a partial output projection; the host sums
the 4 per-core partials of each batch.

Schedule (v2): the kernel is globally software-pipelined so the Activation
engine (exp) and the PE (matmuls) overlap for the whole kernel, not just the
middle. V is projected DIRECTLY into [k, h] layout for both head-pairs at
once (lhsT = x chunk, rhs = Wv-all), killing the PE-transpose round trip.
Pair-A attention runs in reverse q-tile order (t=3..0) so the first exp can
issue ~6us in (tile 3 only needs the last quarter of K/V); pair-B runs
forward (t=0..3) so the kernel ends on its smallest tile. All remaining
projection and out-projection work is chopped into ~1-2us fill units and
paced into the attention chunk stream, keeping the PE dense (p-state ramp)
while ACT chews exps.

Layouts (all matmuls contract on the partition dim):
  qT/kT [h, p]   <- Wpair.T @ xT          (pair-packed: 2x64=128 rows)
  vp    [k, h+1] <- xT_chunk.T @ Wv_all   (direct; ones column -> denom)
  sT    [k, q]   <- kT_chunk.T @ qT       (2 heads via PE quadrant packing)
  e     [k, q]   =  exp(sT / 8) (* strict-lower mask on diagonal blocks)
  z'T   [h+1,q]  <- vp.T @ e              (ones row gives softmax denom d)
  out   [q, m]   <- zn_pair[hh,q].T @ Wo_pair[hh,m]  (accumulated over pairs)

The inverted mask keeps only k > q, so only k-chunks with ki >= 4t are
computed for a 512-wide q-tile t, and boundary chunks are narrowed to width
128*(j+1). Softmax uses shift=0 (scores are O(1); masked entries are exactly
zeroed, never exp(-1e10)). The fully-masked row q=2047 (reference gives
uniform attention) is computed exactly on the host and overwritten.
"""

import sys

for _p in ("/opt/trn_rl_repo",):
    if _p not in sys.path:
        sys.path.insert(0, _p)

from contextlib import ExitStack

import ml_dtypes
import numpy as np

B, P, M, N, H = 2, 2048, 1024, 16, 64
PAIRS = 2          # head pairs per core
QT = 512           # q tile width
NQT = P // QT      # 4 q tiles
KC = 128           # k chunk
NKC = P // KC      # 16 k chunks
NMC = 8            # m chunks of 128
BF16 = ml_dtypes.bfloat16

_CACHE = {}


def _build(reps=1):
    import concourse.bass as bass
    import concourse.tile as tile
    from concourse import bacc, mybir

    BF = mybir.dt.bfloat16
    F32 = mybir.dt.float32
    AF = mybir.ActivationFunctionType

    nc = bacc.Bacc("TRN2", target_bir_lowering=False, debug=False, num_devices=8)

    xT_d = nc.dram_tensor("xT", [M, P], BF, kind="ExternalInput").ap()
    wq_d = nc.dram_tensor("wq", [PAIRS, 128, NMC * 128], BF, kind="ExternalInput").ap()
    wk_d = nc.dram_tensor("wk", [PAIRS, 128, NMC * 128], BF, kind="ExternalInput").ap()
    wv_d = nc.dram_tensor("wv", [128, NMC, 4 * H], BF, kind="ExternalInput").ap()
    wo_d = nc.dram_tensor("wo", [PAIRS, 128, M], BF, kind="ExternalInput").ap()
    mask_d = nc.dram_tensor("mask", [KC, KC], BF, kind="ExternalInput").ap()
    out_d = nc.dram_tensor("out", [P, M], BF, kind="ExternalOutput").ap()

    with tile.TileContext(nc) as tc, ExitStack() as ctx:
        persist = ctx.enter_context(tc.tile_pool(name="persist", bufs=1))
        work = ctx.enter_context(tc.tile_pool(name="work", bufs=1))
        outp = ctx.enter_context(tc.tile_pool(name="outp", bufs=4))
        psum = ctx.enter_context(tc.tile_pool(name="psum", bufs=2, space="PSUM"))
        psum_s = ctx.enter_context(tc.tile_pool(name="psum_s", bufs=2, space="PSUM"))
        psum_z = ctx.enter_context(tc.tile_pool(name="psum_z", bufs=2, space="PSUM"))

        for _rep in range(reps):
            if _rep:
                tc.strict_bb_all_engine_barrier()
            _emit_body(nc, tc, bass, mybir, BF, F32, AF,
                       persist, work, outp, psum, psum_s, psum_z,
                       xT_d, wq_d, wk_d, wv_d, wo_d, mask_d, out_d, _rep)

    nc.compile()
    return nc


def _emit_body(nc, tc, bass, mybir, BF, F32, AF,
               persist, work, outp, psum, psum_s, psum_z,
               xT_d, wq_d, wk_d, wv_d, wo_d, mask_d, out_d, rep):
    # ---- input DMAs: x by q-tile DESC (tile 3 feeds the first attention
    # work); weights interleaved on the scalar queue in need-order ----
    w_sb = {}
    for nm, d, pr in (("wk", wk_d, 0), ("wq", wq_d, 0), ("wk", wk_d, 1),
                      ("wq", wq_d, 1)):
        t = persist.tile([128, NMC, 128], BF, tag=f"{nm}{pr}", name=f"{nm}{pr}")
        w_sb[nm, pr] = t

    wv_sb = persist.tile([128, NMC, 4 * H], BF, tag="wv", name="wv")
    xT = persist.tile([128, NMC, P], BF, tag="xT", name="xT")
    xT_r = xT_d.rearrange("(mo mi) p -> mi mo p", mi=128)
    mask = persist.tile([KC, KC], BF, tag="mask", name="mask")
    wo_sb = []
    for pr in range(PAIRS):
        t = persist.tile([128, M], BF, tag=f"wo{pr}", name=f"wo{pr}")
        wo_sb.append(t)

    def wdma(nm, pr, d):
        nc.scalar.dma_start(
            w_sb[nm, pr][:].rearrange("p mo h -> p (mo h)"), d[pr])

    wdma("wq", 0, wq_d)
    wdma("wk", 0, wk_d)
    nc.sync.dma_start(xT[:, 0:4, bass.ts(3, QT)], xT_r[:, 0:4, bass.ts(3, QT)])
    nc.sync.dma_start(xT[:, 4:8, bass.ts(3, QT)], xT_r[:, 4:8, bass.ts(3, QT)])
    nc.scalar.dma_start(wv_sb[:].rearrange("p mo h -> p (mo h)"),
                        wv_d.rearrange("p mo h -> p (mo h)"))
    nc.sync.dma_start(xT[:, :, bass.ts(2, QT)], xT_r[:, :, bass.ts(2, QT)])
    nc.sync.dma_start(w_sb["wk", 1][:].rearrange("p mo h -> p (mo h)"),
                      wk_d[1])
    nc.sync.dma_start(xT[:, :, bass.ts(1, QT)], xT_r[:, :, bass.ts(1, QT)])
    nc.sync.dma_start(w_sb["wq", 1][:].rearrange("p mo h -> p (mo h)"),
                      wq_d[1])
    nc.sync.dma_start(xT[:, :, bass.ts(0, QT)], xT_r[:, :, bass.ts(0, QT)])
    nc.sync.dma_start(mask[:], mask_d[:])
    for pr in range(PAIRS):
        nc.sync.dma_start(wo_sb[pr][:], wo_d[pr])

    # ---- persistent state ----
    qT, kT = [], []
    for pr in range(PAIRS):
        qT.append(persist.tile([128, P], BF, tag=f"qT{pr}", name=f"qT{pr}"))
        kT.append(persist.tile([128, P], BF, tag=f"kT{pr}", name=f"kT{pr}"))
    # vp carries 64 ones-columns after the 64 v-columns: the AV matmul then
    # emits the softmax denominator d replicated across 64 psum partitions
    # for free (out-partitions don't add matmul time), so no broadcast op is
    # ever needed. The 128-column lhsT also qualifies for FWL.
    vp = persist.tile([128, NKC, PAIRS, 2, 2 * H], BF, tag="vp", name="vp")
    nc.vector.memset(vp[:, :, :, :, H : 2 * H], 1.0)
    zn = [persist.tile([128, P], BF, tag=f"zn{pr}", name=f"zn{pr}")
          for pr in range(PAIRS)]

    # ---- projection units ----
    def proj_kq(pr, nm, ptile, eng="vector", c0=0, cw=QT):
        """qT/kT for cols [c0, c0+cw) of one 512-wide p-tile of one pair."""
        w = w_sb[nm, pr]
        dst = qT[pr] if nm == "wq" else kT[pr]
        p0 = ptile * QT + c0
        ps = psum.tile([128, QT], F32, tag="acc", name="acc")
        for mi in range(NMC):
            nc.tensor.matmul(
                ps[:, 0:cw],
                w[:, mi, :],
                xT[:, mi, p0 : p0 + cw],
                start=(mi == 0),
                stop=(mi == NMC - 1),
            )
        if eng == "scalar":
            nc.scalar.copy(dst[:, p0 : p0 + cw], ps[:, 0:cw])
        else:
            nc.vector.tensor_copy(dst[:, p0 : p0 + cw], ps[:, 0:cw])

    def proj_v(ki):
        """vp for one 128-wide k-chunk, all 4 heads: [k, (pr hh h)] direct."""
        ps = psum.tile([128, QT], F32, tag="acc", name="acc")
        for mi in range(NMC):
            nc.tensor.matmul(
                ps[:, 0 : 4 * H],
                xT[:, mi, bass.ts(ki, KC)],
                wv_sb[:, mi, :],
                start=(mi == 0),
                stop=(mi == NMC - 1),
            )
        nc.vector.tensor_copy(
            vp[:, ki, :, :, 0:H],
            ps[:, 0 : 4 * H].rearrange("p (pr hh h) -> p pr hh h", pr=PAIRS, hh=2),
        )

    # ---- out-projection unit (one 128-row q-chunk) ----
    def out_proj(qc):
        po = [psum.tile([128, QT], F32, tag="acc", name="acc") for _ in range(2)]
        for pr in range(PAIRS):
            for ms in range(2):
                nc.tensor.matmul(
                    po[ms][:],
                    zn[pr][:, bass.ts(qc, 128)],
                    wo_sb[pr][:, bass.ts(ms, QT)],
                    start=(pr == 0),
                    stop=(pr == PAIRS - 1),
                )
        o = outp.tile([128, M], BF, tag="o", name="o")
        nc.scalar.copy(o[:, bass.ts(0, QT)], po[0][:])
        nc.vector.tensor_copy(o[:, bass.ts(1, QT)], po[1][:])
        nc.sync.dma_start(out_d[bass.ts(qc, 128), :], o[:])

    # ---- attention tile with paced fill units ----
    def attn_tile(pr, t, fills=(), end_fills=()):
        fills = list(fills)
        nchunks = NKC - 4 * t
        state = [0, 0]  # chunks retired, fills issued

        def pump():
            state[0] += 1
            want = min(state[0] * len(fills) // nchunks, len(fills))
            while state[1] < want:
                fills[state[1]]()
                state[1] += 1

        pz = [psum_z.tile([128, QT], F32, tag="psZ", name="psZ") for _ in range(2)]

        def emit_scores(ki):
            j = ki - 4 * t
            width = min(KC * (j + 1), QT)
            ps_s = psum_s.tile([128, 2 * QT], F32, tag="psS", name="psS")
            e = work.tile([128, 2 * QT], BF, tag="e", name="e", bufs=8)
            for hh in range(2):
                rows = slice(64 * hh, 64 * hh + 64)
                nc.tensor.matmul(
                    ps_s[:, QT * hh : QT * hh + width],
                    kT[pr][rows, bass.ts(ki, KC)],
                    qT[pr][rows, t * QT : t * QT + width],
                    start=True,
                    stop=True,
                    tile_position=(64 * hh, 0),
                )
            if width == QT:  # full chunk: one exp covers both heads
                nc.scalar.activation(e[:], ps_s[:], AF.Exp, scale=0.125)
            else:
                for hh in range(2):
                    nc.scalar.activation(
                        e[:, QT * hh : QT * hh + width],
                        ps_s[:, QT * hh : QT * hh + width],
                        AF.Exp,
                        scale=0.125,
                    )
            if j < 4:  # diagonal block: strict-lower mask
                for hh in range(2):
                    nc.vector.tensor_mul(
                        e[:, QT * hh + KC * j : QT * hh + KC * (j + 1)],
                        e[:, QT * hh + KC * j : QT * hh + KC * (j + 1)],
                        mask[:],
                    )
            if ki == NKC - 1 and t == NQT - 1:
                # keep d(q=2047) nonzero; that row is host-computed
                for hh in range(2):
                    nc.vector.memset(
                        e[:, QT * hh + width - 1 : QT * hh + width], 1.0
                    )
            return e, width

        def emit_av(ki, e, width):
            for hh in range(2):
                nc.tensor.matmul(
                    pz[hh][:, :width],
                    vp[:, ki, pr, hh, :],
                    e[:, QT * hh : QT * hh + width],
                    start=(ki == NKC - 1),
                    stop=(ki == 4 * t),
                )

        # chunks run in DESCENDING ki so the widest chunk (ki=15, width 512)
        # opens the psum accumulation group: every later AV accumulates into
        # fully-initialized columns (no per-byte mixed overwrite/accumulate).
        # software pipeline: scores(next) emitted before AV(cur) so the PE
        # stream never blocks on exp(cur) with ready scores work behind it
        pend = None
        for ki in range(NKC - 1, 4 * t - 1, -1):
            cur = (ki, *emit_scores(ki))
            if pend is not None:
                emit_av(*pend)
                pump()
            pend = cur
        emit_av(*pend)
        pump()
        while state[1] < len(fills):  # stragglers
            fills[state[1]]()
            state[1] += 1
        for f in end_fills:  # pads the PE during the normalize chain below
            f()

        # normalize: zn = z * (1/d); pz rows [0:H) = z, rows [H:2H) = d
        # replicated 64x by the vp ones-columns. Evacuate both halves to
        # SBUF right away (frees the pz slots in ~1.5us; the custom-DVE
        # reciprocal also needs a partition-0 SBUF input).
        for hh in range(2):
            zraw = work.tile([H, QT], F32, tag=f"zr{hh}", name=f"zr{hh}",
                             bufs=2)
            draw = work.tile([H, QT], F32, tag=f"dr{hh}", name=f"dr{hh}",
                             bufs=2)
            nc.vector.tensor_copy(zraw[:], pz[hh][0:H, :])
            nc.vector.tensor_copy(draw[:], pz[hh][H : 2 * H, :])
            rb = work.tile([H, QT], F32, tag=f"rb{hh}", name=f"rb{hh}",
                           bufs=2)
            nc.vector.reciprocal_approx_fast(rb[:], draw[:])
            nc.vector.tensor_mul(
                zn[pr][64 * hh : 64 * hh + 64, bass.ts(t, QT)],
                zraw[:],
                rb[:],
            )

    # ---- global schedule ----
    # Pair-sequential tile order (all A tiles, then all B tiles): an evenly
    # interleaved A/B order measured ~18% SLOWER — saturating all engines
    # simultaneously appears to trip the package power throttle (P0), while
    # this bunched order alternates which engine is hot.
    # Prefix: exactly what scores(ki=15) of tile A3 needs, in need-order, so
    # the first exp issues as early as possible.
    A, Bp = 0, 1
    proj_kq(A, "wq", 3, eng="scalar")
    proj_kq(A, "wk", 3, eng="scalar", c0=3 * KC, cw=KC)  # k-chunk 15 first
    proj_kq(A, "wk", 3, eng="scalar", c0=0, cw=3 * KC)
    for ki in (15, 14, 13, 12):
        proj_v(ki)

    attn_tile(A, 3, fills=[
        lambda: proj_kq(A, "wk", 2, eng="scalar"),
        lambda: proj_kq(A, "wq", 2, eng="scalar"),
        lambda: proj_v(11), lambda: proj_v(10),
    ], end_fills=[lambda: proj_v(9), lambda: proj_v(8)])
    attn_tile(A, 2, fills=[
        lambda: proj_kq(A, "wk", 1),
        lambda: proj_kq(A, "wq", 1),
        lambda: proj_v(7), lambda: proj_v(6),
    ], end_fills=[lambda: proj_v(5), lambda: proj_v(4)])
    attn_tile(A, 1, fills=[
        lambda: proj_kq(A, "wk", 0),
        lambda: proj_kq(A, "wq", 0),
        lambda: proj_v(3), lambda: proj_v(2),
        lambda: proj_v(1), lambda: proj_v(0),
        lambda: proj_kq(Bp, "wk", 3),
        lambda: proj_kq(Bp, "wk", 2),
    ], end_fills=[
        lambda: proj_kq(Bp, "wk", 1),
        lambda: proj_kq(Bp, "wk", 0),
    ])
    attn_tile(A, 0, fills=[
        lambda: proj_kq(Bp, "wq", 0),
    ], end_fills=[lambda: proj_kq(Bp, "wq", 1)])
    attn_tile(Bp, 0, fills=[
        lambda: proj_kq(Bp, "wq", 2),
    ], end_fills=[lambda: proj_kq(Bp, "wq", 3)])
    attn_tile(Bp, 1,
              fills=[lambda: out_proj(0), lambda: out_proj(1)],
              end_fills=[lambda: out_proj(2)])
    attn_tile(Bp, 2,
              fills=[lambda: out_proj(3), lambda: out_proj(4)],
              end_fills=[lambda: out_proj(5)])
    attn_tile(Bp, 3,
              fills=[lambda: out_proj(6), lambda: out_proj(7)],
              end_fills=[lambda qc=qc: out_proj(qc) for qc in range(8, 12)])
    for qc in range(12, 16):
        out_proj(qc)


def kernel(
    x,
    kernel_query,
    bias_query,
    kernel_key,
    bias_key,
    kernel_value,
    bias_value,
    kernel_out,
    bias_out,
):
    from concourse.bass_utils import run_bass_kernel_spmd

    if "nc" not in _CACHE:
        _CACHE["nc"] = _build()
    nc = _CACHE["nc"]

    x = np.asarray(x, np.float32)
    wq = np.asarray(kernel_query, np.float32)
    wk = np.asarray(kernel_key, np.float32)
    wv = np.asarray(kernel_value, np.float32)
    wo = np.asarray(kernel_out, np.float32)
    bo = np.asarray(bias_out, np.float32)
    bv = np.asarray(bias_value, np.float32)

    mask = np.tril(np.ones((KC, KC), np.float32), -1).astype(BF16)
    xT = [np.ascontiguousarray(x[b].T).astype(BF16) for b in range(B)]

    def pack_w(w, grp):  # [N, M, H] -> [PAIRS, 128, NMC*128] (mi-major relayout)
        a = np.stack(
            [
                np.concatenate([w[4 * grp + 2 * pr], w[4 * grp + 2 * pr + 1]], axis=1)
                for pr in range(PAIRS)
            ]
        )  # [PAIRS, (mo mi), h]
        a = a.reshape(PAIRS, NMC, 128, 128).transpose(0, 2, 1, 3)
        return np.ascontiguousarray(a.reshape(PAIRS, 128, NMC * 128)).astype(BF16)

    def pack_wv(w, grp):  # [N, M, H] -> [128, NMC, 4H]: rows mi, cols (pr hh h)
        cat = np.concatenate([w[4 * grp + j] for j in range(4)], axis=1)  # [M, 4H]
        return np.ascontiguousarray(
            cat.reshape(NMC, 128, 4 * H).transpose(1, 0, 2)
        ).astype(BF16)

    def pack_wo(w, grp):  # [N, H, M] -> [PAIRS, 128, M]
        return np.stack(
            [
                np.concatenate([w[4 * grp + 2 * pr], w[4 * grp + 2 * pr + 1]], axis=0)
                for pr in range(PAIRS)
            ]
        ).astype(BF16)

    in_maps = []
    for c in range(8):
        b, grp = c // 4, c % 4
        in_maps.append(
            {
                "xT": xT[b],
                "wq": pack_w(wq, grp),
                "wk": pack_w(wk, grp),
                "wv": pack_wv(wv, grp),
                "wo": pack_wo(wo, grp),
                "mask": mask,
            }
        )

    _CACHE["last_in_maps"] = in_maps
    res = run_bass_kernel_spmd(nc, in_maps, core_ids=list(range(8)))
    _CACHE["last_result"] = res

    out = np.zeros((B, P, M), np.float32)
    for c in range(8):
        out[c // 4] += res.results[c]["out"].astype(np.float32)

    # exact host-side bias fold: sum_n bv_n @ Wo_n + bo (zero for this spec)
    bias_fold = np.einsum("nh,nhm->m", bv, wo) + bo
    out += bias_fold[None, None, :]

    # the fully-masked last query row attends uniformly: z_n = mean_k v_n[k]
    for b in range(B):
        xmean = x[b].mean(axis=0)
        row = sum(
            (xmean @ wv[n] + bv[n]) @ wo[n] for n in range(N)
        ) + bo
        out[b, P - 1, :] = row
    return out


if __name__ == "__main__":
    rng = np.random.default_rng(0)
    ins = {
        "x": rng.standard_normal((B, P, M), np.float32) * 1.0,
        "kernel_query": 0.02 * rng.standard_normal((N, M, H), np.float32),
        "bias_query": np.zeros((N, H), np.float32),
        "kernel_key": 0.02 * rng.standard_normal((N, M, H), np.float32),
        "bias_key": np.zeros((N, H), np.float32),
        "kernel_value": 0.02 * rng.standard_normal((N, M, H), np.float32),
        "bias_value": np.zeros((N, H), np.float32),
        "kernel_out": 0.02 * rng.standard_normal((N, H, M), np.float32),
        "bias_out": np.zeros((M,), np.float32),
    }
    o = kernel(**ins)
    print("kernel out", o.shape, o.dtype, np.abs(o).max())


# revision 34
# speedup vs baseline: 1.2062x; 1.2062x over previous
"""Multi-head self-attention (inverted causal mask) on 8 Trainium2 cores.

Problem: B=2, P=2048 seq, M=1024 model dim, N=16 heads, H=64 head dim.
Sharding: data-parallel on batch (2) x tensor-parallel on heads (4 groups
of 4 heads) = 8 cores. Each core computes, for its batch b and its 4 heads,
the full attention pipeline and a partial output projection; the host sums
the 4 per-core partials of each batch.

Schedule (v2): the kernel is globally software-pipelined so the Activation
engine (exp) and the PE (matmuls) overlap for the whole kernel, not just the
middle. V is projected DIRECTLY into [k, h] layout for both head-pairs at
once (lhsT = x chunk, rhs = Wv-all), killing the PE-transpose round trip.
Pair-A attention runs in reverse q-tile order (t=3..0) so the first exp can
issue ~6us in (tile 3 only needs the last quarter of K/V); pair-B runs
forward (t=0..3) so the kernel ends on its smallest tile. All remaining
projection and out-projection work is chopped into ~1-2us fill units and
paced into the attention chunk stream, keeping the PE dense (p-state ramp)
while ACT chews exps.

Layouts (all matmuls contract on the partition dim):
  qT/kT [h, p]   <- Wpair.T @ xT          (pair-packed: 2x64=128 rows)
  vp    [k, h+1] <- xT_chunk.T @ Wv_all   (direct; ones column -> denom)
  sT    [k, q]   <- kT_chunk.T @ qT       (2 heads via PE quadrant packing)
  e     [k, q]   =  exp(sT / 8) (* strict-lower mask on diagonal blocks)
  z'T   [h+1,q]  <- vp.T @ e              (ones row gives softmax denom d)
  out   [q, m]   <- zn_pair[hh,q].T @ Wo_pair[hh,m]  (accumulated over pairs)

The inverted mask keeps only k > q, so only k-chunks with ki >= 4t are
computed for a 512-wide q-tile t, and boundary chunks are narrowed to width
128*(j+1). Softmax uses shift=0 (scores are O(1); masked entries are exactly
zeroed, never exp(-1e10)). The fully-masked row q=2047 (reference gives
uniform attention) is computed exactly on the host and overwritten.
"""

import sys

for _p in ("/opt/trn_rl_repo",):
    if _p not in sys.path:
        sys.path.insert(0, _p)

from contextlib import ExitStack

import ml_dtypes
import numpy as np

B, P, M, N, H = 2, 2048, 1024, 16, 64
PAIRS = 2          # head pairs per core
QT = 512           # q tile width
NQT = P // QT      # 4 q tiles
KC = 128           # k chunk
NKC = P // KC      # 16 k chunks
NMC = 8            # m chunks of 128
BF16 = ml_dtypes.bfloat16

_CACHE = {}


def _build(reps=1):
    import concourse.bass as bass
    import concourse.tile as tile
    from concourse import bacc, mybir

    BF = mybir.dt.bfloat16
    F32 = mybir.dt.float32
    AF = mybir.ActivationFunctionType

    nc = bacc.Bacc("TRN2", target_bir_lowering=False, debug=False, num_devices=8)

    xT_d = nc.dram_tensor("xT", [M, P], BF, kind="ExternalInput").ap()
    wq_d = nc.dram_tensor("wq", [PAIRS, 128, NMC * 128], BF, kind="ExternalInput").ap()
    wk_d = nc.dram_tensor("wk", [PAIRS, 128, NMC * 128], BF, kind="ExternalInput").ap()
    wv_d = nc.dram_tensor("wv", [128, NMC, 4 * H], BF, kind="ExternalInput").ap()
    wo_d = nc.dram_tensor("wo", [PAIRS, 128, M], BF, kind="ExternalInput").ap()
    mask_d = nc.dram_tensor("mask", [KC, KC], BF, kind="ExternalInput").ap()
    out_d = nc.dram_tensor("out", [P, M], BF, kind="ExternalOutput").ap()

    with tile.TileContext(nc) as tc, ExitStack() as ctx:
        persist = ctx.enter_context(tc.tile_pool(name="persist", bufs=1))
        work = ctx.enter_context(tc.tile_pool(name="work", bufs=1))
        outp = ctx.enter_context(tc.tile_pool(name="outp", bufs=4))
        psum = ctx.enter_context(tc.tile_pool(name="psum", bufs=2, space="PSUM"))
        psum_s = ctx.enter_context(tc.tile_pool(name="psum_s", bufs=2, space="PSUM"))
        psum_z = ctx.enter_context(tc.tile_pool(name="psum_z", bufs=2, space="PSUM"))

        for _rep in range(reps):
            if _rep:
                tc.strict_bb_all_engine_barrier()
            _emit_body(nc, tc, bass, mybir, BF, F32, AF,
                       persist, work, outp, psum, psum_s, psum_z,
                       xT_d, wq_d, wk_d, wv_d, wo_d, mask_d, out_d, _rep)

    nc.compile()
    return nc


def _emit_body(nc, tc, bass, mybir, BF, F32, AF,
               persist, work, outp, psum, psum_s, psum_z,
               xT_d, wq_d, wk_d, wv_d, wo_d, mask_d, out_d, rep):
    # ---- input DMAs: x by q-tile DESC (tile 3 feeds the first attention
    # work); weights interleaved on the scalar queue in need-order ----
    w_sb = {}
    for nm, d, pr in (("wk", wk_d, 0), ("wq", wq_d, 0), ("wk", wk_d, 1),
                      ("wq", wq_d, 1)):
        t = persist.tile([128, NMC, 128], BF, tag=f"{nm}{pr}", name=f"{nm}{pr}")
        w_sb[nm, pr] = t

    wv_sb = persist.tile([128, NMC, 4 * H], BF, tag="wv", name="wv")
    xT = persist.tile([128, NMC, P], BF, tag="xT", name="xT")
    xT_r = xT_d.rearrange("(mo mi) p -> mi mo p", mi=128)
    mask = persist.tile([KC, KC], BF, tag="mask", name="mask")
    wo_sb = []
    for pr in range(PAIRS):
        t = persist.tile([128, M], BF, tag=f"wo{pr}", name=f"wo{pr}")
        wo_sb.append(t)

    def wdma(nm, pr, d):
        nc.scalar.dma_start(
            w_sb[nm, pr][:].rearrange("p mo h -> p (mo h)"), d[pr])

    wdma("wq", 0, wq_d)
    wdma("wk", 0, wk_d)
    nc.sync.dma_start(xT[:, 0:4, bass.ts(3, QT)], xT_r[:, 0:4, bass.ts(3, QT)])
    nc.sync.dma_start(xT[:, 4:8, bass.ts(3, QT)], xT_r[:, 4:8, bass.ts(3, QT)])
    nc.sync.dma_start(wv_sb[:].rearrange("p mo h -> p (mo h)"),
                      wv_d.rearrange("p mo h -> p (mo h)"))
    nc.sync.dma_start(xT[:, :, bass.ts(2, QT)], xT_r[:, :, bass.ts(2, QT)])
    nc.sync.dma_start(w_sb["wk", 1][:].rearrange("p mo h -> p (mo h)"),
                      wk_d[1])
    nc.sync.dma_start(xT[:, :, bass.ts(1, QT)], xT_r[:, :, bass.ts(1, QT)])
    nc.sync.dma_start(w_sb["wq", 1][:].rearrange("p mo h -> p (mo h)"),
                      wq_d[1])
    nc.sync.dma_start(xT[:, :, bass.ts(0, QT)], xT_r[:, :, bass.ts(0, QT)])
    nc.sync.dma_start(mask[:], mask_d[:])
    for pr in range(PAIRS):
        nc.sync.dma_start(wo_sb[pr][:], wo_d[pr])

    # ---- persistent state ----
    qT, kT = [], []
    for pr in range(PAIRS):
        qT.append(persist.tile([128, P], BF, tag=f"qT{pr}", name=f"qT{pr}"))
        kT.append(persist.tile([128, P], BF, tag=f"kT{pr}", name=f"kT{pr}"))
    # vp carries 64 ones-columns after the 64 v-columns: the AV matmul then
    # emits the softmax denominator d replicated across 64 psum partitions
    # for free (out-partitions don't add matmul time), so no broadcast op is
    # ever needed. The 128-column lhsT also qualifies for FWL.
    vp = persist.tile([128, NKC, PAIRS, 2, 2 * H], BF, tag="vp", name="vp")
    nc.vector.memset(vp[:, :, :, :, H : 2 * H], 1.0)
    zn = [persist.tile([128, P], BF, tag=f"zn{pr}", name=f"zn{pr}")
          for pr in range(PAIRS)]

    # ---- projection units ----
    def proj_kq(pr, nm, ptile, eng="vector", c0=0, cw=QT):
        """qT/kT for cols [c0, c0+cw) of one 512-wide p-tile of one pair."""
        w = w_sb[nm, pr]
        dst = qT[pr] if nm == "wq" else kT[pr]
        p0 = ptile * QT + c0
        ps = psum.tile([128, QT], F32, tag="acc", name="acc")
        for mi in range(NMC):
            nc.tensor.matmul(
                ps[:, 0:cw],
                w[:, mi, :],
                xT[:, mi, p0 : p0 + cw],
                start=(mi == 0),
                stop=(mi == NMC - 1),
            )
        if eng == "scalar":
            nc.scalar.copy(dst[:, p0 : p0 + cw], ps[:, 0:cw])
        else:
            nc.vector.tensor_copy(dst[:, p0 : p0 + cw], ps[:, 0:cw])

    def proj_v(ki):
        """vp for one 128-wide k-chunk, all 4 heads: [k, (pr hh h)] direct."""
        ps = psum.tile([128, QT], F32, tag="acc", name="acc")
        for mi in range(NMC):
            nc.tensor.matmul(
                ps[:, 0 : 4 * H],
                xT[:, mi, bass.ts(ki, KC)],
                wv_sb[:, mi, :],
                start=(mi == 0),
                stop=(mi == NMC - 1),
            )
        nc.vector.tensor_copy(
            vp[:, ki, :, :, 0:H],
            ps[:, 0 : 4 * H].rearrange("p (pr hh h) -> p pr hh h", pr=PAIRS, hh=2),
        )

    # ---- out-projection unit (one 128-row q-chunk) ----
    def out_proj(qc):
        po = [psum.tile([128, QT], F32, tag="acc", name="acc") for _ in range(2)]
        for pr in range(PAIRS):
            for ms in range(2):
                nc.tensor.matmul(
                    po[ms][:],
                    zn[pr][:, bass.ts(qc, 128)],
                    wo_sb[pr][:, bass.ts(ms, QT)],
                    start=(pr == 0),
                    stop=(pr == PAIRS - 1),
                )
        o = outp.tile([128, M], BF, tag="o", name="o")
        nc.scalar.copy(o[:, bass.ts(0, QT)], po[0][:])
        nc.vector.tensor_copy(o[:, bass.ts(1, QT)], po[1][:])
        nc.sync.dma_start(out_d[bass.ts(qc, 128), :], o[:])

    # ---- attention tile with paced fill units ----
    def attn_tile(pr, t, fills=(), end_fills=()):
        fills = list(fills)
        nchunks = NKC - 4 * t
        state = [0, 0]  # chunks retired, fills issued

        def pump():
            state[0] += 1
            want = min(state[0] * len(fills) // nchunks, len(fills))
            while state[1] < want:
                fills[state[1]]()
                state[1] += 1

        pz = [psum_z.tile([128, QT], F32, tag="psZ", name="psZ") for _ in range(2)]

        def emit_scores(ki):
            j = ki - 4 * t
            width = min(KC * (j + 1), QT)
            ps_s = psum_s.tile([128, 2 * QT], F32, tag="psS", name="psS")
            e = work.tile([128, 2 * QT], BF, tag="e", name="e", bufs=8)
            for hh in range(2):
                rows = slice(64 * hh, 64 * hh + 64)
                nc.tensor.matmul(
                    ps_s[:, QT * hh : QT * hh + width],
                    kT[pr][rows, bass.ts(ki, KC)],
                    qT[pr][rows, t * QT : t * QT + width],
                    start=True,
                    stop=True,
                    tile_position=(64 * hh, 0),
                )
            if width == QT:  # full chunk: one exp covers both heads
                nc.scalar.activation(e[:], ps_s[:], AF.Exp, scale=0.125)
            else:
                for hh in range(2):
                    nc.scalar.activation(
                        e[:, QT * hh : QT * hh + width],
                        ps_s[:, QT * hh : QT * hh + width],
                        AF.Exp,
                        scale=0.125,
                    )
            if j < 4:  # diagonal block: strict-lower mask
                for hh in range(2):
                    nc.vector.tensor_mul(
                        e[:, QT * hh + KC * j : QT * hh + KC * (j + 1)],
                        e[:, QT * hh + KC * j : QT * hh + KC * (j + 1)],
                        mask[:],
                    )
            if ki == NKC - 1 and t == NQT - 1:
                # keep d(q=2047) nonzero; that row is host-computed
                for hh in range(2):
                    nc.vector.memset(
                        e[:, QT * hh + width - 1 : QT * hh + width], 1.0
                    )
            return e, width

        def emit_av(ki, e, width):
            for hh in range(2):
                nc.tensor.matmul(
                    pz[hh][:, :width],
                    vp[:, ki, pr, hh, :],
                    e[:, QT * hh : QT * hh + width],
                    start=(ki == NKC - 1),
                    stop=(ki == 4 * t),
                )

        # chunks run in DESCENDING ki so the widest chunk (ki=15, width 512)
        # opens the psum accumulation group: every later AV accumulates into
        # fully-initialized columns (no per-byte mixed overwrite/accumulate).
        # software pipeline: scores(next) emitted before AV(cur) so the PE
        # stream never blocks on exp(cur) with ready scores work behind it
        pend = None
        for ki in range(NKC - 1, 4 * t - 1, -1):
            cur = (ki, *emit_scores(ki))
            if pend is not None:
                emit_av(*pend)
                pump()
            pend = cur
        emit_av(*pend)
        pump()
        while state[1] < len(fills):  # stragglers
            fills[state[1]]()
            state[1] += 1
        for f in end_fills:  # pads the PE during the normalize chain below
            f()

        # normalize: zn = z * (1/d); pz rows [0:H) = z, rows [H:2H) = d
        # replicated 64x by the vp ones-columns. Evacuate both halves to
        # SBUF right away (frees the pz slots in ~1.5us; the custom-DVE
        # reciprocal also needs a partition-0 SBUF input).
        for hh in range(2):
            zraw = work.tile([H, QT], F32, tag=f"zr{hh}", name=f"zr{hh}",
                             bufs=2)
            draw = work.tile([H, QT], F32, tag=f"dr{hh}", name=f"dr{hh}",
                             bufs=2)
            nc.vector.tensor_copy(zraw[:], pz[hh][0:H, :])
            nc.vector.tensor_copy(draw[:], pz[hh][H : 2 * H, :])
            rb = work.tile([H, QT], F32, tag=f"rb{hh}", name=f"rb{hh}",
                           bufs=2)
            nc.vector.reciprocal_approx_fast(rb[:], draw[:])
            nc.vector.tensor_mul(
                zn[pr][64 * hh : 64 * hh + 64, bass.ts(t, QT)],
                zraw[:],
                rb[:],
            )

    # ---- global schedule ----
    # Pair-sequential tile order (all A tiles, then all B tiles): an evenly
    # interleaved A/B order measured ~18% SLOWER — saturating all engines
    # simultaneously appears to trip the package power throttle (P0), while
    # this bunched order alternates which engine is hot.
    # Prefix: exactly what scores(ki=15) of tile A3 needs, in need-order, so
    # the first exp issues as early as possible.
    A, Bp = 0, 1
    proj_kq(A, "wq", 3, eng="scalar")
    proj_kq(A, "wk", 3, eng="scalar", c0=3 * KC, cw=KC)  # k-chunk 15 first
    proj_kq(A, "wk", 3, eng="scalar", c0=0, cw=3 * KC)
    for ki in (15, 14, 13, 12):
        proj_v(ki)

    attn_tile(A, 3, fills=[
        lambda: proj_kq(A, "wk", 2, eng="scalar"),
        lambda: proj_kq(A, "wq", 2, eng="scalar"),
        lambda: proj_v(11), lambda: proj_v(10),
    ], end_fills=[lambda: proj_v(9), lambda: proj_v(8)])
    attn_tile(A, 2, fills=[
        lambda: proj_kq(A, "wk", 1),
        lambda: proj_kq(A, "wq", 1),
        lambda: proj_v(7), lambda: proj_v(6),
    ], end_fills=[lambda: proj_v(5), lambda: proj_v(4)])
    attn_tile(A, 1, fills=[
        lambda: proj_kq(A, "wk", 0),
        lambda: proj_kq(A, "wq", 0),
        lambda: proj_v(3), lambda: proj_v(2),
        lambda: proj_v(1), lambda: proj_v(0),
        lambda: proj_kq(Bp, "wk", 3),
        lambda: proj_kq(Bp, "wk", 2),
    ], end_fills=[
        lambda: proj_kq(Bp, "wk", 1),
        lambda: proj_kq(Bp, "wk", 0),
    ])
    attn_tile(A, 0, fills=[
        lambda: proj_kq(Bp, "wq", 0),
    ], end_fills=[lambda: proj_kq(Bp, "wq", 1)])
    attn_tile(Bp, 0, fills=[
        lambda: proj_kq(Bp, "wq", 2),
    ], end_fills=[lambda: proj_kq(Bp, "wq", 3)])
    attn_tile(Bp, 1,
              fills=[lambda: out_proj(0), lambda: out_proj(1)],
              end_fills=[lambda: out_proj(2)])
    attn_tile(Bp, 2,
              fills=[lambda: out_proj(3), lambda: out_proj(4)],
              end_fills=[lambda: out_proj(5)])
    attn_tile(Bp, 3,
              fills=[lambda: out_proj(6), lambda: out_proj(7)],
              end_fills=[lambda qc=qc: out_proj(qc) for qc in range(8, 12)])
    for qc in range(12, 16):
        out_proj(qc)


def kernel(
    x,
    kernel_query,
    bias_query,
    kernel_key,
    bias_key,
    kernel_value,
    bias_value,
    kernel_out,
    bias_out,
):
    from concourse.bass_utils import run_bass_kernel_spmd

    if "nc" not in _CACHE:
        _CACHE["nc"] = _build()
    nc = _CACHE["nc"]

    x = np.asarray(x, np.float32)
    wq = np.asarray(kernel_query, np.float32)
    wk = np.asarray(kernel_key, np.float32)
    wv = np.asarray(kernel_value, np.float32)
    wo = np.asarray(kernel_out, np.float32)
    bo = np.asarray(bias_out, np.float32)
    bv = np.asarray(bias_value, np.float32)

    mask = np.tril(np.ones((KC, KC), np.float32), -1).astype(BF16)
    xT = [np.ascontiguousarray(x[b].T).astype(BF16) for b in range(B)]

    def pack_w(w, grp):  # [N, M, H] -> [PAIRS, 128, NMC*128] (mi-major relayout)
        a = np.stack(
            [
                np.concatenate([w[4 * grp + 2 * pr], w[4 * grp + 2 * pr + 1]], axis=1)
                for pr in range(PAIRS)
            ]
        )  # [PAIRS, (mo mi), h]
        a = a.reshape(PAIRS, NMC, 128, 128).transpose(0, 2, 1, 3)
        return np.ascontiguousarray(a.reshape(PAIRS, 128, NMC * 128)).astype(BF16)

    def pack_wv(w, grp):  # [N, M, H] -> [128, NMC, 4H]: rows mi, cols (pr hh h)
        cat = np.concatenate([w[4 * grp + j] for j in range(4)], axis=1)  # [M, 4H]
        return np.ascontiguousarray(
            cat.reshape(NMC, 128, 4 * H).transpose(1, 0, 2)
        ).astype(BF16)

    def pack_wo(w, grp):  # [N, H, M] -> [PAIRS, 128, M]
        return np.stack(
            [
                np.concatenate([w[4 * grp + 2 * pr], w[4 * grp + 2 * pr + 1]], axis=0)
                for pr in range(PAIRS)
            ]
        ).astype(BF16)

    in_maps = []
    for c in range(8):
        b, grp = c // 4, c % 4
        in_maps.append(
            {
                "xT": xT[b],
                "wq": pack_w(wq, grp),
                "wk": pack_w(wk, grp),
                "wv": pack_wv(wv, grp),
                "wo": pack_wo(wo, grp),
                "mask": mask,
            }
        )

    _CACHE["last_in_maps"] = in_maps
    res = run_bass_kernel_spmd(nc, in_maps, core_ids=list(range(8)))
    _CACHE["last_result"] = res

    out = np.zeros((B, P, M), np.float32)
    for c in range(8):
        out[c // 4] += res.results[c]["out"].astype(np.float32)

    # exact host-side bias fold: sum_n bv_n @ Wo_n + bo (zero for this spec)
    bias_fold = np.einsum("nh,nhm->m", bv, wo) + bo
    out += bias_fold[None, None, :]

    # the fully-masked last query row attends uniformly: z_n = mean_k v_n[k]
    for b in range(B):
        xmean = x[b].mean(axis=0)
        row = sum(
            (xmean @ wv[n] + bv[n]) @ wo[n] for n in range(N)
        ) + bo
        out[b, P - 1, :] = row
    return out


if __name__ == "__main__":
    rng = np.random.default_rng(0)
    ins = {
        "x": rng.standard_normal((B, P, M), np.float32) * 1.0,
        "kernel_query": 0.02 * rng.standard_normal((N, M, H), np.float32),
        "bias_query": np.zeros((N, H), np.float32),
        "kernel_key": 0.02 * rng.standard_normal((N, M, H), np.float32),
        "bias_key": np.zeros((N, H), np.float32),
        "kernel_value": 0.02 * rng.standard_normal((N, M, H), np.float32),
        "bias_value": np.zeros((N, H), np.float32),
        "kernel_out": 0.02 * rng.standard_normal((N, H, M), np.float32),
        "bias_out": np.zeros((M,), np.float32),
    }
    o = kernel(**ins)
    print("kernel out", o.shape, o.dtype, np.abs(o).max())
